# revision 20
# baseline (speedup 1.0000x reference)
"""Trainium2 Bass kernel v4 for dual-branch spatial attention.

v4 = v3 with the PSUM-WAR critical chain broken:
  - v3's spa/spd score tiles were single-buffered, so each score matmul
    WAR-waited on the exp (ACT/DVE) of the chunk one cycle back:
    chain = MM 375 + sync + exp 1226 + sync ~= 1766ns per 2 chunks.
  - v4 uses ONE shared score pool, bufs=3 ([128,1024] f32 = 2 banks each,
    6 banks): score(k+3) waits exp(k), which is ~2 cycles stale -> no
    stall.  The accumulator shrinks to two independent [65,512] tags
    (1 bank each; h0/h1 query halves), copied out per-half right after
    that half's last PV matmul.  PSUM = 6 + 2 = 8 banks exactly.
  - warmup trimmed 48 -> 20 matmuls (it only needs to bridge the DMA
    head so HAM sees continuous PE busy; the rest was delaying real
    work).
  - everything else (host-hoisted projections, duo row-tiling scores,
    ones-column denominator, AD exp split, host division) as in v3.

PSUM budget (8 banks): sp 3x[128,1024] (6) + accm [128,512] (1) +
accd [128,512] (1).

v5 on top: PV restructured from 4 to 3 slots per chunk pair via column
tiling.  Each chunk's 64 channels ride ONE N=512 slot (two 64-col tiles:
h0 queries -> acc rows 0:64, h1 -> rows 64:128 of one bank), and the
softmax denominators for a whole pair ride ONE 4-way col-tiled slot
(1-col ones weights at partitions 0/32/64/96 of the den bank).  The
ones column of v3/v4 (65-wide PV weight) is gone, so each pex column
streams through the array once for channels instead of twice.
"""

import os
import sys

import numpy as np

for _p in ("/opt/trn_rl_repo", "/root/.axon_site/_ro/trn_rl_repo"):
    if os.path.isdir(_p) and _p not in sys.path:
        sys.path.insert(0, _p)

B, C, H, W = 4, 64, 64, 64
HW = H * W            # 4096
QS = HW // 2          # 2048 query rows per core
QB = 1024             # query block (phase width)
KC = 128              # key chunk
NKC = HW // KC        # 32 chunks
NCORES = 8
NPH = (QS // QB) * 2  # 4 phases (qb, br)

_GRAPH_CACHE = {}

_EXPC = (0.99903364, 0.25053222, 0.03244013, 0.0025659)


_EXP_OP = None


def _ensure_exp_op():
    """Register a fused sq(sq(horner3)) custom DVE op so the vector engine
    can serve as a second exp engine alongside ScalarE."""
    global _EXP_OP
    if _EXP_OP is not None:
        return _EXP_OP
    from concourse import dve_ops
    from concourse.dve_spec import (
        C0, C1, C2, C3, Spec, Src0, _spill_c3_to_src1, lower, sq,
    )
    from concourse.dve_uop import DveOpSpec

    body = _spill_c3_to_src1(
        sq(sq(((Src0 * C3 + C2) * Src0 + C1) * Src0 + C0))
    )

    def _ref(in0, in1, c0, c1, c2):
        x = in0.astype(np.float32)
        p = ((x * in1 + c2) * x + c1) * x + c0
        return (p * p) * (p * p)

    spec = Spec(body=body, reference=_ref)
    name = "EXP_POLY4_ANT"
    row = dve_ops._CUSTOM_DVE_ROW_BASE + len(dve_ops.OPS)
    shas = {}
    for ver in ("v3", "v4"):
        try:
            tmp = DveOpSpec(name=name, opcode=row, uops=lower(spec, ver=ver),
                            rd1_en=True)
            shas[ver] = tmp.sha(ver)
        except Exception:
            pass
    op = dve_ops.DveOp(name=name, spec=spec, subdim=False, uops_sha=shas)
    dve_ops.OPS.append(op)
    dve_ops._SUB_OPCODE_FOR_NAME[name] = row
    dve_ops.CUSTOM_DVE_SPECS[name] = spec
    _EXP_OP = op
    return op


def _build_graph(skew=5, duo=True):
    from concourse import bacc, bass, mybir, tile

    exp_op = _ensure_exp_op()

    f32 = mybir.dt.float32
    bf16 = mybir.dt.bfloat16
    Exp = mybir.ActivationFunctionType.Exp

    nc = bacc.Bacc(None)
    kpd = nc.declare_dram_parameter("kpd", [128, HW], bf16, isOutput=False)
    k1pd = nc.declare_dram_parameter("k1pd", [128, HW], bf16, isOutput=False)
    vtd = nc.declare_dram_parameter("vtd", [128, NKC * 128], bf16, isOutput=False)
    ya = nc.declare_dram_parameter("ya", [128, QS], bf16, isOutput=False)
    xq = nc.declare_dram_parameter("xq", [128, QS], bf16, isOutput=False)
    outm = nc.declare_dram_parameter("outm", [128, NPH, 512], f32, isOutput=True)
    outd = nc.declare_dram_parameter("outd", [97, NPH, 512], f32, isOutput=True)

    with tile.TileContext(nc) as tc:
        with tc.tile_pool(name="singles", bufs=1) as singles:
            kp_sb = singles.tile([128, HW], bf16)
            k1p_sb = singles.tile([128, HW], bf16)
            vt_sb = singles.tile([128, NKC, 128], bf16)
            ya_sb = singles.tile([128, QS], bf16)
            xq_sb = singles.tile([128, QS], bf16)
            c3_sb = singles.tile([128, 1], f32)
            ones_sb = singles.tile([128, 1], bf16)
            nc.vector.memset(ones_sb[:, :], 1.0)

            # DMA order = first-use order.  Only the two EARLIEST pieces ride
            # the scalar queue: every DMA_DIRECT2D on an engine queue blocks
            # that engine's later instructions (FIFO), and ScalarE must be
            # free for exp by ~9us.  Everything else streams on sync, which
            # has no compute to block.  vt pieces go early: walrus coalesces
            # the first compute block's DMA waits into one semaphore wait, so
            # a late vt piece stalls the first score matmul (observed +2us).
            nc.sync.dma_start(out=kp_sb[:, 0:512], in_=kpd[:, 0:512])
            nc.scalar.dma_start(out=ya_sb[:, 0:512], in_=ya[:, 0:512])
            nc.sync.dma_start(out=ya_sb[:, 512:1024], in_=ya[:, 512:1024])
            nc.scalar.dma_start(out=kp_sb[:, 512:1024], in_=kpd[:, 512:1024])
            nc.sync.dma_start(out=vt_sb[:, 0:8, :], in_=vtd[:, 0:8 * 128])
            nc.sync.dma_start(out=vt_sb[:, 8:16, :], in_=vtd[:, 8 * 128:16 * 128])
            nc.sync.dma_start(out=kp_sb[:, 1024:1536], in_=kpd[:, 1024:1536])
            nc.sync.dma_start(out=kp_sb[:, 1536:2048], in_=kpd[:, 1536:2048])
            nc.sync.dma_start(out=kp_sb[:, 2048:2560], in_=kpd[:, 2048:2560])
            nc.sync.dma_start(out=vt_sb[:, 16:24, :], in_=vtd[:, 16 * 128:24 * 128])
            nc.sync.dma_start(out=vt_sb[:, 24:32, :], in_=vtd[:, 24 * 128:32 * 128])
            nc.sync.dma_start(out=kp_sb[:, 2560:3072], in_=kpd[:, 2560:3072])
            nc.sync.dma_start(out=kp_sb[:, 3072:3584], in_=kpd[:, 3072:3584])
            nc.sync.dma_start(out=kp_sb[:, 3584:4096], in_=kpd[:, 3584:4096])
            nc.sync.dma_start(out=xq_sb[:, 0:1024], in_=xq[:, 0:1024])
            nc.sync.dma_start(out=k1p_sb[:, 0:2048], in_=k1pd[:, 0:2048])
            nc.sync.dma_start(out=k1p_sb[:, 2048:4096], in_=k1pd[:, 2048:4096])
            nc.sync.dma_start(out=ya_sb[:, 1024:2048], in_=ya[:, 1024:2048])
            nc.sync.dma_start(out=xq_sb[:, 1024:2048], in_=xq[:, 1024:2048])

            nc.vector.memset(c3_sb[:, :], _EXPC[3])
            # dependency-free dummy exp pulls the ACT table load early
            warm_sb = singles.tile([128, 1], f32)
            nc.scalar.activation(out=warm_sb[:, :], in_=c3_sb[:, :], func=Exp)

            # PE warm-up: the HAM clock gate releases 2.4GHz only after a
            # ~3.4us fully-busy window, and whether that happens early is
            # otherwise a per-run dice roll (runs measure bimodally at MM
            # p50 379 vs 454).  Burn the DMA-bound head on dense dummy
            # matmuls so the upshift is deterministic.  20 is enough to
            # bridge until kp[:, :512] + ya[:, :1024] have landed.
            wmt = singles.tile([128, 64], bf16)
            nc.vector.memset(wmt[:, :], 1.0)
            with tc.tile_pool(name="wpsum", bufs=1, space="PSUM") as wpool:
                wp = wpool.tile([64, 64], f32, tag="w")
                for _ in range(20):
                    nc.tensor.matmul(
                        wp[:, :], lhsT=wmt[:, 0:64], rhs=wmt[:, :],
                        start=True, stop=True,
                    )

            from collections import deque

            def rows(h):
                return slice(64 * h, 64 * (h + 1)) if duo else slice(0, 64)

            with tc.tile_pool(name="spp", bufs=3, space="PSUM") as spp, \
                 tc.tile_pool(name="accp", bufs=1, space="PSUM") as accp, \
                 tc.tile_pool(name="pexa", bufs=8) as pexa, \
                 tc.tile_pool(name="pexd", bufs=6) as pexd, \
                 tc.tile_pool(name="osb", bufs=2) as opool:

                accs = {}
                pending = deque()
                scored = {}

                def drain(lim):
                    # PVs go in CHUNK PAIRS (two channel slots + one 4-way
                    # col-tiled den slot).  Hold a phase's first pair until a
                    # few of its chunks have been scored so the acc-WAR wait
                    # on the previous phase's evacuation copies resolves off
                    # the PE's critical path.
                    while len(pending) >= max(lim, 2):
                        hph, hkc, _ = pending[0]
                        if hkc == 0 and scored.get(hph, 0) < 2 and lim > 1:
                            break
                        p0 = pending.popleft()
                        p1 = pending.popleft()
                        emit_pair(p0, p1)

                def emit_pair(p0, p1):
                    ph, kc0, pex0 = p0
                    _, kc1, pex1 = p1
                    am, ad = accs[ph]
                    first = kc0 == 0
                    last = kc1 == NKC - 1
                    # channel slots: each chunk's 64 channels as two 64-col
                    # tiles -- h0 queries land on acc rows 0:64, h1 queries
                    # on rows 64:128 of the SAME bank (col tiling).  NOTE:
                    # start=True clears has_written only for the PARTITIONS
                    # that matmul writes (measured on HW; not the whole
                    # bank), so each region's first matmul must carry it.
                    for j, (kc, ppex) in enumerate(((kc0, pex0), (kc1, pex1))):
                        nc.tensor.matmul(
                            am[0:64, :], lhsT=vt_sb[:, kc, 0:64],
                            rhs=ppex[:, 0:512],
                            start=(first and j == 0), stop=False,
                            skip_group_check=True,
                        )
                        nc.tensor.matmul(
                            am[64:128, :], lhsT=vt_sb[:, kc, 64:128],
                            rhs=ppex[:, 512:1024],
                            start=(first and j == 0), stop=(last and j == 1),
                            skip_group_check=True,
                        )
                    # den slot: 4 concurrent 1-col ones-weight tiles at
                    # partitions 0/32/64/96 (col tiling 4x, 8 XBUSes), one
                    # per (chunk, query-half) of the pair
                    for j, (pos, ppex, cols) in enumerate((
                        (0, pex0, slice(0, 512)),
                        (32, pex0, slice(512, 1024)),
                        (64, pex1, slice(0, 512)),
                        (96, pex1, slice(512, 1024)),
                    )):
                        nc.tensor.matmul(
                            ad[pos:pos + 1, :], lhsT=ones_sb[:, 0:1],
                            rhs=ppex[:, cols],
                            start=first, stop=(last and j == 3),
                            skip_group_check=True,
                            # explicit: the auto-derive calls
                            # out.base_partition(), whose rust getter rejects
                            # 96 (known quirk -- pass tile_position instead)
                            tile_position=(0, pos),
                        )
                    if last:
                        om = opool.tile([128, 512], f32, tag="om")
                        od = opool.tile([97, 512], f32, tag="od")
                        if ph == NPH - 1:
                            # final phase: split copies across both engines
                            # and both DMA queues for the shortest tail
                            nc.scalar.copy(out=om[:, :], in_=am[:, :])
                            nc.vector.tensor_copy(out=od[:, :], in_=ad[0:97, :])
                            nc.scalar.dma_start(out=outm[:, ph, :], in_=om[:, :])
                            nc.sync.dma_start(out=outd[:, ph, :], in_=od[:, :])
                        else:
                            # both copies on ScalarE: DVE is the busier exp
                            # engine (1226ns/op vs 1114), so it must not
                            # also pay evacuation copies
                            nc.scalar.copy(out=om[:, :], in_=am[:, :])
                            nc.scalar.copy(out=od[:, :], in_=ad[0:97, :])
                            nc.sync.dma_start(out=outm[:, ph, :], in_=om[:, :])
                            nc.sync.dma_start(out=outd[:, ph, :], in_=od[:, :])
                        del accs[ph]

                def emit_exp(sp, use_dve, split=False):
                    # split=True: two 512-wide halves so the trailing PV can
                    # start after half the exp latency (used at the drain)
                    cols = [slice(0, QB)] if not split else [
                        slice(0, 512), slice(512, QB)]
                    if use_dve:
                        pex = pexd.tile([128, QB], bf16, tag="pexd", name="pex")
                        for c in cols:
                            nc.vector._custom_dve(
                                exp_op, out=pex[:, c], in0=sp[:, c],
                                in1=c3_sb[:, :], s0=_EXPC[0], s1=_EXPC[1],
                                imm2=_EXPC[2],
                            )
                    else:
                        pex = pexa.tile([128, QB], bf16, tag="pexa", name="pex")
                        for c in cols:
                            nc.scalar.activation(out=pex[:, c], in_=sp[:, c], func=Exp)
                    return pex

                # engine pattern: (ACT, DVE) per chunk pair.  ACT also pays
                # the two per-phase evacuation copies, which evens out its
                # lower per-op cost (1114ns vs DVE 1226ns).
                pair_sched = ["AD"] * 16
                pair_sched_last = ["AD"] * 15 + ["DA"]
                for qb in range(QS // QB):
                    for br in range(2):
                        ph = qb * 2 + br
                        kp = kp_sb if br == 0 else k1p_sb
                        qsrc = ya_sb if br == 0 else xq_sb
                        q0 = qb * QB
                        last_ph = ph == NPH - 1
                        accs[ph] = (
                            accp.tile([128, 512], f32, tag="am", name="accm"),
                            accp.tile([128, 512], f32, tag="ad", name="accd"),
                        )
                        for m in range(NKC // 2):
                            kinds = (pair_sched_last if last_ph else pair_sched)[m]
                            lim = 2 if (last_ph and m >= NKC // 2 - 3) else skew
                            drain(lim)
                            for s in range(2):
                                kc = 2 * m + s
                                sp = spp.tile(
                                    [128, QB], f32, tag="sp", name="sp",
                                )
                                for h in range(QB // 512):
                                    nc.tensor.matmul(
                                        sp[:, h * 512:(h + 1) * 512],
                                        lhsT=kp[rows(h), kc * KC:(kc + 1) * KC],
                                        rhs=qsrc[rows(h), q0 + h * 512:q0 + (h + 1) * 512],
                                        start=True,
                                        stop=True,
                                    )
                                pex = emit_exp(
                                    sp, kinds[s] == "D",
                                    split=last_ph and m >= NKC // 2 - 2,
                                )
                                pending.append((ph, kc, pex[:, :]))
                                scored[ph] = scored.get(ph, 0) + 1
                while pending:
                    p0 = pending.popleft()
                    p1 = pending.popleft()
                    emit_pair(p0, p1)
    if not nc.is_finalized():
        nc.finalize()
    return nc


def _get_graph(**kw):
    key = tuple(sorted(kw.items()))
    if key not in _GRAPH_CACHE:
        _GRAPH_CACHE[key] = _build_graph(**kw)
    return _GRAPH_CACHE[key]


def _prep_in_maps(inputs):
    f = lambda k: np.asarray(inputs[k], dtype=np.float32)
    x, y = f("x"), f("y")
    Wq, Wk, Wv = f("Wq"), f("Wk"), f("Wv")
    Wq1, Wk1 = f("Wq1"), f("Wk1")

    xr = x.reshape(B, C, HW)
    yr = y.reshape(B, C, HW)

    d = np.float64
    G = (Wq.astype(d).T @ Wk.astype(d)).astype(np.float32)    # (64,64)
    G1 = (Wq1.astype(d).T @ Wk1.astype(d)).astype(np.float32)

    import ml_dtypes

    b16 = ml_dtypes.bfloat16

    def dup(a):
        """duplicate the 64 rows into partitions 64:128 (PE row-tile T8)"""
        return np.concatenate([a, a], axis=0).astype(b16)

    in_maps = []
    for b in range(B):
        kp = G @ xr[b]                    # (64, HW)
        k1p = G1 @ xr[b]
        vt = xr[b].T @ Wv.T               # (HW, 64)
        vtp = np.empty((128, NKC, 128), np.float32)
        vtp[:, :, 0:64] = vt.reshape(NKC, 128, 64).transpose(1, 0, 2)
        vtp[:, :, 64:128] = vtp[:, :, 0:64]   # dup for the h1 col-tile
        kpd = dup(kp)
        k1pd = dup(k1p)
        vtd = vtp.astype(b16).reshape(128, NKC * 128)
        for qh in range(2):
            q0 = qh * QS
            in_maps.append(
                {
                    "kpd": kpd,
                    "k1pd": k1pd,
                    "vtd": vtd,
                    "ya": dup(yr[b][:, q0: q0 + QS]),
                    "xq": dup(xr[b][:, q0: q0 + QS]),
                }
            )
    return in_maps


def _postprocess(results):
    full = np.empty((B, C, HW), np.float32)
    for i in range(NCORES):
        b, qh = i // 2, i % 2
        om = results[i]["outm"]           # (128, NPH, 512)
        od = results[i]["outd"]           # (97, NPH, 512)
        for qb in range(QS // QB):
            acc = None
            for br in range(2):
                ph = qb * 2 + br
                # rows 0:64 = h0 queries, 64:128 = h1; dens: even chunks at
                # den rows 0 (h0) / 32 (h1), odd chunks at 64 (h0) / 96 (h1)
                n = np.concatenate([om[0:64, ph, :], om[64:128, ph, :]], 1)
                d = np.concatenate(
                    [od[0, ph, :] + od[64, ph, :],
                     od[32, ph, :] + od[96, ph, :]], 0)
                t = n / d[None, :]
                acc = t if acc is None else acc + t
            full[b, :, qh * QS + qb * QB: qh * QS + (qb + 1) * QB] = acc
    return full.reshape(B, C, H, W)


def _execute(inputs, trace=False, **graph_kw):
    from concourse.bass_utils import run_bass_kernel_spmd

    nc = _get_graph(**graph_kw)
    in_maps = _prep_in_maps(inputs)
    res = run_bass_kernel_spmd(
        nc, in_maps, core_ids=list(range(NCORES)), trace=trace
    )
    return _postprocess(res.results), res


def kernel(**inputs):
    out, _ = _execute(inputs)
    return out

